# revision 15
# baseline (speedup 1.0000x reference)
"""PotNet GNN message-passing kernel for 8 Trainium2 NeuronCores (Bass/Tile).

Strategy: edges sorted by dst and sharded so core c owns nodes
[c*2500, (c+1)*2500) and ALL edges targeting them (no cross-core aggregation
needed). Edge MLPs run locally in fp16 (weight-stationary matmuls on
transposed activations); the two BatchNorms need global batch stats, done
with tiny AllReduce collectives. Aggregation (segment_sum) is a one-hot
indicator matmul into per-node-block PSUM accumulators. The x[dst] gather is
avoided entirely: edges are grouped by 128-node blocks, so the dst
contribution of layer 1 is (x_blk @ W1a) expanded per-edge with the same
one-hot indicator (transposed) on the PE.
"""
import numpy as np

import concourse.bass as bass
import concourse.bacc as bacc
import concourse.mybir as mybir
import concourse.tile as tile
import concourse.bass_utils as bass_utils
from concourse.masks import make_identity

P = 128          # partitions / chunk size (edges)
F = 256          # feature dim
FH = 2           # feature halves (F // P)
GC = 4           # chunks per compute group
GROUP = P * GC   # 512 edges per group
EPS = 1e-5

N_NODES = 20000
N_CORES = 8

DT = mybir.dt.float16
F32 = mybir.dt.float32
NPDT = np.float16

_cache = {}


# ---------------------------------------------------------------- host prep
def _prep(x, edge_index, edge_attr, n_nodes, n_cores):
    E = edge_index.shape[1]
    NPC = n_nodes // n_cores          # nodes per core
    NB = -(-NPC // P)                 # node blocks per core
    src = np.asarray(edge_index[0], np.int64)
    dst = np.asarray(edge_index[1], np.int64)

    order = np.argsort(dst, kind="stable")
    dst_s = dst[order]
    src_s = src[order]

    core_of = dst_s // NPC
    nl = dst_s - core_of * NPC        # node-local id
    b_of = nl // P                    # block within core
    dloc = (nl - b_of * P)            # position within block

    gid = core_of * NB + b_of
    cnts = np.bincount(gid, minlength=n_cores * NB).reshape(n_cores, NB)
    K_b = np.maximum(1, -(-cnts.max(axis=0) // P)).astype(np.int64)  # chunks/block
    C0 = int(K_b.sum())
    K_b[NB - 1] += (-C0) % GC
    C = int(K_b.sum())
    G = C // GC
    base_b = np.zeros(NB, np.int64)
    base_b[1:] = np.cumsum(K_b)[:-1]
    E_PAD = C * P

    # rank of each edge within its (core, block) group
    group_starts = np.zeros(n_cores * NB + 1, np.int64)
    group_starts[1:] = np.cumsum(np.bincount(gid, minlength=n_cores * NB))
    rank = np.arange(E) - group_starts[gid]
    slot = (base_b[b_of] + rank // P) * P + rank % P   # slot within core's arrays

    Vx = n_nodes + 1
    x16 = np.vstack([np.asarray(x, np.float32).astype(NPDT),
                     np.zeros((1, F), NPDT)])          # row n_nodes = zeros
    ea16 = np.asarray(edge_attr, np.float32).astype(NPDT)[order]

    per_core = []
    for c in range(n_cores):
        m = core_of == c
        sl = slot[m]
        ea_sh = np.zeros((E_PAD, F), NPDT)
        ea_sh[sl] = ea16[m]
        row, col = sl % P, sl // P
        src_i = np.full((P, C), n_nodes, np.int32)
        dlc = np.full((P, C), -1.0, NPDT)
        src_i[row, col] = src_s[m]
        dlc[row, col] = dloc[m].astype(NPDT)
        xr = np.zeros((NB * P, F), np.float32)
        xr[:NPC] = np.asarray(x, np.float32)[c * NPC:(c + 1) * NPC]
        per_core.append(dict(ea=ea_sh, srci=src_i, dstloc=dlc,
                             xres=xr, xblk=xr.astype(NPDT)))
    meta = dict(E=E, NPC=NPC, NB=NB, C=C, G=G, Vx=Vx, E_PAD=E_PAD,
                K_b=K_b, base_b=base_b)
    return per_core, x16, meta


def _col2(v):
    """[F] vector -> [P, FH] f32, feature f = c*P + p at [p, c]."""
    return np.asarray(v, np.float32).reshape(FH, P).T.copy()


def _silu64(v):
    return v / (1.0 + np.exp(-v))


def _chunk_runs(chunks, block_of_chunk):
    """Split chunk ids into runs of equal block: [(b, i0, i1)]."""
    runs = []
    i0 = 0
    for i in range(1, len(chunks) + 1):
        if i == len(chunks) or \
                block_of_chunk[chunks[i]] != block_of_chunk[chunks[i0]]:
            runs.append((int(block_of_chunk[chunks[i0]]), i0, i))
            i0 = i
    return runs


# ---------------------------------------------------------------- device build
def _build(meta, n_cores, n_nodes, sim1=False, cfg=None):
    cfg = cfg or dict(mm=2, tp=4, wk=4, comb=4, agg=3)
    BL1 = cfg.get('l1', 2); BL2 = cfg.get('l2', 1); BTP = cfg.get('tp', 2)
    BMM = cfg.get('mm', 0)  # if set, share one psum pool/tag for l1+l2
    BWK = cfg.get('wk', 3); BCB = cfg.get('comb', 2)
    BAG = cfg.get('agg', 2)
    NB, C, G, Vx = meta["NB"], meta["C"], meta["G"], meta["Vx"]
    K_b, base_b = meta["K_b"], meta["base_b"]
    E_PAD, E = meta["E_PAD"], meta["E"]
    block_of_chunk = np.repeat(np.arange(NB), K_b)

    inv_E = 1.0 / float(E)
    inv_N = 1.0 / float(n_nodes)

    nc = bacc.Bacc("TRN2", target_bir_lowering=False, debug=False,
                   enable_asserts=True, num_devices=1 if sim1 else n_cores)

    ea = nc.dram_tensor("ea", [E_PAD, F], DT, kind="ExternalInput")
    srci = nc.dram_tensor("srci", [P, C], mybir.dt.int32, kind="ExternalInput")
    dstloc = nc.dram_tensor("dstloc", [P, C], DT, kind="ExternalInput")
    xt = nc.dram_tensor("xt", [Vx, F], DT, kind="ExternalInput")
    xres = nc.dram_tensor("xres", [NB * P, F], F32, kind="ExternalInput")
    xblk = nc.dram_tensor("xblk", [NB * P, F], DT, kind="ExternalInput")
    w1f = nc.dram_tensor("w1f", [3 * F, F], DT, kind="ExternalInput")
    w2f = nc.dram_tensor("w2f", [F, F], DT, kind="ExternalInput")
    w1m = nc.dram_tensor("w1m", [3 * F, F], DT, kind="ExternalInput")
    w2m = nc.dram_tensor("w2m", [F, F], DT, kind="ExternalInput")
    pnames = ["gint", "bint", "gbn", "bbn", "corr1", "corr2"]
    pdram = {n: nc.dram_tensor(n, [P, FH], F32, kind="ExternalInput")
             for n in pnames}
    iota = nc.dram_tensor("iota", [P, P], DT, kind="ExternalInput")
    out = nc.dram_tensor("out", [NB * P, F], F32, kind="ExternalOutput")

    SCR = 4 * GROUP   # combined scratch row: [gp0|gp1|mp0|mp1]

    with tile.TileContext(nc) as tc:
        with tc.tile_pool(name="const", bufs=1) as cst, \
             tc.tile_pool(name="work", bufs=BWK) as wk, \
             tc.tile_pool(name="aggsb", bufs=1) as aggpool, \
             tc.tile_pool(name="dram", bufs=1, space="DRAM") as dram:

            # ---------------- resident constants
            w1t = {}
            w2t = {}
            for nm, drt in (("f", w1f), ("m", w1m)):
                w1t[nm] = []
                for k in range(6):
                    t = cst.tile([P, F], DT, tag=f"w1{nm}{k}", name=f"w1{nm}{k}")
                    nc.sync.dma_start(out=t[:], in_=drt[k * P:(k + 1) * P, :])
                    w1t[nm].append(t)
            for nm, drt in (("f", w2f), ("m", w2m)):
                w2t[nm] = []
                for k in range(FH):
                    t = cst.tile([P, F], DT, tag=f"w2{nm}{k}", name=f"w2{nm}{k}")
                    nc.sync.dma_start(out=t[:], in_=drt[k * P:(k + 1) * P, :])
                    w2t[nm].append(t)
            pr = {}
            for n in pnames:
                t = cst.tile([P, FH], F32, tag=f"p_{n}", name=f"p_{n}")
                nc.sync.dma_start(out=t[:], in_=pdram[n][:])
                pr[n] = t
            iot = cst.tile([P, P], DT, tag="iota")
            nc.sync.dma_start(out=iot[:], in_=iota[:])
            ident = cst.tile([P, P], DT, tag="ident")
            make_identity(nc, ident[:])
            ident32 = cst.tile([P, P], F32, tag="ident32")
            make_identity(nc, ident32[:])
            dl_sb = cst.tile([P, C], DT, tag="dstloc")
            nc.sync.dma_start(out=dl_sb[:], in_=dstloc[:])
            si_sb = cst.tile([P, C], mybir.dt.int32, tag="srci")
            nc.sync.dma_start(out=si_sb[:], in_=srci[:])
            ones_c = cst.tile([P, 1], F32, tag="ones_c")
            nc.gpsimd.memset(ones_c[:], 1.0)
            ones_r = cst.tile([1, P], F32, tag="ones_r")
            nc.gpsimd.memset(ones_r[:], 1.0)
            s1 = cst.tile([P, FH], F32, tag="s1")
            nc.gpsimd.memset(s1[:], 0.0)
            s2 = cst.tile([P, FH], F32, tag="s2")
            nc.gpsimd.memset(s2[:], 0.0)

            scr = dram.tile([G, P, SCR], DT, tag="scr")

            # ---------------- PASS A
            if BMM:
                ctxA = tc.tile_pool(name="ps_mm", bufs=BMM, space="PSUM")
                ps_l1 = ctxA.__enter__()
                ctxA2 = None
                ps_l2 = ps_l1
            else:
                ctxA = tc.tile_pool(name="ps_l1", bufs=BL1, space="PSUM")
                ps_l1 = ctxA.__enter__()
                ctxA2 = tc.tile_pool(name="ps_l2", bufs=BL2, space="PSUM")
                ps_l2 = ctxA2.__enter__()
            ctxA3 = tc.tile_pool(name="ps_tp", bufs=BTP, space="PSUM")
            ps_tp = ctxA3.__enter__()

            # lazily-computed per-block xW1a tables (dst part of layer 1)
            xw1a = {"f": [None] * NB, "m": [None] * NB}

            def ensure_xw1a(b):
                if xw1a["f"][b] is not None:
                    return
                xb = wk.tile([P, F], DT, tag="xb", name=f"xb{b}")
                nc.sync.dma_start(out=xb[:], in_=xblk[b * P:(b + 1) * P, :])
                tpx = ps_tp.tile([P, FH * P], DT, tag="tp", name=f"tpx{b}")
                for kc in range(FH):
                    nc.tensor.transpose(out=tpx[:, kc * P:(kc + 1) * P],
                                        in_=xb[:, kc * P:(kc + 1) * P],
                                        identity=ident[:])
                xbT = wk.tile([P, FH, P], DT, tag="xbT", name=f"xbT{b}")
                nc.vector.tensor_copy(
                    out=xbT[:], in_=tpx[:].rearrange("p (c q) -> p c q", c=FH))
                for br in ("f", "m"):
                    psx = ps_l1.tile([P, F], F32, tag="mm" if BMM else "l1",
                                     name=f"psx{b}{br}")
                    for kc in range(FH):
                        nc.tensor.matmul(out=psx[:], lhsT=xbT[:, kc],
                                         rhs=w1t[br][kc][:],
                                         start=(kc == 0), stop=(kc == FH - 1))
                    xw = wk.tile([P, F], DT, tag=f"xw1a{br}", name=f"xw{b}{br}")
                    nc.vector.tensor_copy(out=xw[:], in_=psx[:])
                    xw1a[br][b] = xw

            for g in range(G):
                e0 = g * GROUP
                chunks = list(range(g * GC, (g + 1) * GC))
                runs = _chunk_runs(chunks, block_of_chunk)
                for b, _, _ in runs:
                    ensure_xw1a(b)

                # x[src] gather + transpose
                gxs = wk.tile([P, GC, F], DT, tag="g_xs")
                for sc in range(GC):
                    nc.gpsimd.indirect_dma_start(
                        out=gxs[:, sc], out_offset=None, in_=xt[:],
                        in_offset=bass.IndirectOffsetOnAxis(
                            ap=si_sb[:, g * GC + sc:g * GC + sc + 1], axis=0))
                hT = []
                for h in range(FH):
                    tp = ps_tp.tile([P, GROUP], DT, tag="tp", name=f"tpxs{h}")
                    for sc in range(GC):
                        nc.tensor.transpose(
                            out=tp[:, sc * P:(sc + 1) * P],
                            in_=gxs[:, sc, h * P:(h + 1) * P],
                            identity=ident[:])
                    tT = wk.tile([P, GROUP], DT, tag=f"T_xs{h}", name=f"T_xs{h}")
                    nc.vector.tensor_copy(out=tT[:], in_=tp[:])
                    hT.append(tT)
                for h in range(FH):
                    tT = wk.tile([P, GROUP], DT, tag=f"T_ea{h}", name=f"T_ea{h}")
                    nc.sync.dma_start(out=tT[:],
                                      in_=ea[e0:e0 + GROUP, h * P:(h + 1) * P],
                                      transpose=True)
                    hT.append(tT)

                # indicator (edge-part) + transposed version for dst expansion
                ind_g = wk.tile([P, GC, P], DT, tag="ind_g")
                for sc in range(GC):
                    nc.vector.tensor_tensor(
                        out=ind_g[:, sc], in0=iot[:],
                        in1=dl_sb[:, g * GC + sc:g * GC + sc + 1].to_broadcast([P, P]),
                        op=mybir.AluOpType.is_equal)
                tpi = ps_tp.tile([P, GROUP], DT, tag="tp", name="tpi")
                for sc in range(GC):
                    nc.tensor.transpose(out=tpi[:, sc * P:(sc + 1) * P],
                                        in_=ind_g[:, sc], identity=ident[:])
                indT = wk.tile([P, GROUP], DT, tag="indT")
                nc.vector.tensor_copy(out=indT[:], in_=tpi[:])

                comb = wk.tile([P, SCR], DT, tag="comb", bufs=BCB)
                for bi_, br in enumerate(("f", "m")):
                    ps = ps_l1.tile([P, 2 * GROUP], F32, tag="mm" if BMM else "l1",
                                    name=f"l1{br}")
                    # src + ea contributions first (no indT dependency)
                    for k in range(4):
                        for j in range(FH):
                            nc.tensor.matmul(
                                out=ps[:, j * GROUP:(j + 1) * GROUP],
                                lhsT=w1t[br][2 + k][:, j * P:(j + 1) * P],
                                rhs=hT[k][:], start=(k == 0), stop=False,
                                skip_group_check=True)
                    # dst contribution via indicator expansion (last)
                    for b, i0, i1 in runs:
                        for j in range(FH):
                            nc.tensor.matmul(
                                out=ps[:, j * GROUP + i0 * P:j * GROUP + i1 * P],
                                lhsT=xw1a[br][b][:, j * P:(j + 1) * P],
                                rhs=indT[:, i0 * P:i1 * P],
                                start=False, stop=True, skip_group_check=True)
                    t1 = wk.tile([P, 2 * GROUP], DT, tag=f"t1{br}",
                                 name=f"t1{br}")
                    for k in range(FH):
                        nc.scalar.activation(
                            t1[:, k * GROUP:(k + 1) * GROUP],
                            ps[:, k * GROUP:(k + 1) * GROUP],
                            mybir.ActivationFunctionType.Silu)
                    ps2 = ps_l2.tile([P, 2 * GROUP], F32, tag="mm" if BMM else "l2",
                                     name=f"l2{br}")
                    for k in range(FH):
                        for j in range(FH):
                            nc.tensor.matmul(
                                out=ps2[:, j * GROUP:(j + 1) * GROUP],
                                lhsT=w2t[br][k][:, j * P:(j + 1) * P],
                                rhs=t1[:, k * GROUP:(k + 1) * GROUP],
                                start=(k == 0), stop=(k == FH - 1))
                    if br == "f":
                        # evict per half with running-sum accumulation (ACT);
                        # sumsq on DVE from the fp16 gp values
                        for j in range(FH):
                            r1 = wk.tile([P, 1], F32, tag=f"r1_{j}",
                                         name=f"r1_{j}")
                            nc.scalar.activation(
                                comb[:, j * GROUP:(j + 1) * GROUP],
                                ps2[:, j * GROUP:(j + 1) * GROUP],
                                mybir.ActivationFunctionType.Identity,
                                accum_out=r1[:])
                            nc.vector.tensor_tensor(
                                out=s1[:, j:j + 1], in0=s1[:, j:j + 1],
                                in1=r1[:], op=mybir.AluOpType.add)
                        sqt = wk.tile([P, 2 * GROUP], DT, tag="sqt", bufs=2)
                        nc.vector.tensor_tensor(
                            out=sqt[:], in0=comb[:, 0:2 * GROUP],
                            in1=comb[:, 0:2 * GROUP], op=mybir.AluOpType.mult)
                        r2 = wk.tile([P, FH], F32, tag="r2")
                        nc.vector.tensor_reduce(
                            out=r2[:],
                            in_=sqt[:].rearrange("p (c q) -> p c q", c=FH),
                            op=mybir.AluOpType.add, axis=mybir.AxisListType.X)
                        nc.vector.tensor_tensor(out=s2[:], in0=s2[:],
                                                in1=r2[:],
                                                op=mybir.AluOpType.add)
                    else:
                        nc.scalar.activation(
                            comb[:, bi_ * 2 * GROUP:(bi_ + 1) * 2 * GROUP],
                            ps2[:],
                            mybir.ActivationFunctionType.Identity)
                nc.scalar.dma_start(out=scr[g], in_=comb[:])

            ctxA3.__exit__(None, None, None)
            if ctxA2 is not None:
                ctxA2.__exit__(None, None, None)
            ctxA.__exit__(None, None, None)

            # ---------------- BN1 stats all-reduce + finalize
            stg = wk.tile([P, 2 * FH], F32, tag="stg")
            nc.vector.tensor_copy(out=stg[:, 0:FH], in_=s1[:])
            nc.vector.tensor_copy(out=stg[:, FH:2 * FH], in_=s2[:])
            cc1_in = dram.tile([P, 2 * FH], F32, tag="cc1_in")
            cc1_out = dram.tile([P, 2 * FH], F32, tag="cc1_out")
            nc.gpsimd.dma_start(out=cc1_in[:], in_=stg[:])
            if sim1:
                nc.gpsimd.dma_start(out=cc1_out[:], in_=cc1_in[:])
            else:
                nc.gpsimd.collective_compute(
                    "AllReduce", mybir.AluOpType.add,
                    replica_groups=[list(range(n_cores))],
                    ins=[cc1_in.opt()], outs=[cc1_out.opt()])
            st_all = wk.tile([P, 2 * FH], F32, tag="st_all")
            nc.sync.dma_start(out=st_all[:], in_=cc1_out[:])

            def bn_finalize(st_ap, corr1, corr2, gam, bet, inv_n, tag):
                """-> (alpha [P,FH], beta [P,FH]) f32 tiles."""
                mean = wk.tile([P, FH], F32, tag=f"{tag}mean", name=f"{tag}mean")
                nc.vector.tensor_tensor(out=mean[:], in0=st_ap[:, 0:FH],
                                        in1=corr1[:], op=mybir.AluOpType.subtract)
                nc.vector.tensor_scalar_mul(mean[:], mean[:], inv_n)
                ex2 = wk.tile([P, FH], F32, tag=f"{tag}ex2", name=f"{tag}ex2")
                nc.vector.tensor_tensor(out=ex2[:], in0=st_ap[:, FH:2 * FH],
                                        in1=corr2[:], op=mybir.AluOpType.subtract)
                nc.vector.tensor_scalar_mul(ex2[:], ex2[:], inv_n)
                var = wk.tile([P, FH], F32, tag=f"{tag}var", name=f"{tag}var")
                nc.vector.tensor_tensor(out=var[:], in0=mean[:], in1=mean[:],
                                        op=mybir.AluOpType.mult)
                nc.vector.tensor_tensor(out=var[:], in0=ex2[:], in1=var[:],
                                        op=mybir.AluOpType.subtract)
                nc.vector.tensor_scalar_add(var[:], var[:], float(EPS))
                sd = wk.tile([P, FH], F32, tag=f"{tag}sd", name=f"{tag}sd")
                nc.scalar.activation(sd[:], var[:],
                                     mybir.ActivationFunctionType.Sqrt)
                rs = wk.tile([P, FH], F32, tag=f"{tag}rs", name=f"{tag}rs")
                nc.vector.reciprocal(rs[:], sd[:])
                al = wk.tile([P, FH], F32, tag=f"{tag}al", name=f"{tag}al")
                nc.vector.tensor_tensor(out=al[:], in0=gam[:], in1=rs[:],
                                        op=mybir.AluOpType.mult)
                be = wk.tile([P, FH], F32, tag=f"{tag}be", name=f"{tag}be")
                nc.vector.tensor_tensor(out=be[:], in0=al[:], in1=mean[:],
                                        op=mybir.AluOpType.mult)
                nc.vector.tensor_tensor(out=be[:], in0=bet[:], in1=be[:],
                                        op=mybir.AluOpType.subtract)
                return al, be

            al1, be1 = bn_finalize(st_all, pr["corr1"], pr["corr2"],
                                   pr["gint"], pr["bint"], inv_E, "bn1")

            # ---------------- PASS B
            zero2 = cst.tile([P, FH], F32, tag="zero2")
            nc.gpsimd.memset(zero2[:], 0.0)
            agg_sb = [None] * NB
            agg_ps = {}
            ctxB = tc.tile_pool(name="ps_tpB", bufs=2, space="PSUM")
            ps_tpB = ctxB.__enter__()
            ctxB2 = tc.tile_pool(name="ps_agg", bufs=BAG, space="PSUM")
            ps_agg = ctxB2.__enter__()
            for g in range(G):
                comb_in = wk.tile([P, SCR], DT, tag="combB", bufs=BCB)
                nc.sync.dma_start(out=comb_in[:], in_=scr[g])
                msgT = []
                for j in range(FH):
                    gt = wk.tile([P, GROUP], DT, tag=f"gate_{j}",
                                 name=f"gate{j}")
                    nc.scalar.activation(
                        gt[:], comb_in[:, j * GROUP:(j + 1) * GROUP],
                        mybir.ActivationFunctionType.Sigmoid,
                        bias=be1[:, j:j + 1], scale=al1[:, j:j + 1])
                    mt = wk.tile([P, GROUP], DT, tag=f"msgT_{j}",
                                 name=f"msgT{j}")
                    nc.vector.tensor_tensor(
                        out=mt[:], in0=gt[:],
                        in1=comb_in[:, (2 + j) * GROUP:(3 + j) * GROUP],
                        op=mybir.AluOpType.mult)
                    msgT.append(mt)
                msg_gb = wk.tile([P, GC, F], DT, tag="msg_gb")
                for j in range(FH):
                    tp = ps_tpB.tile([P, GROUP], DT, tag="tpB", name=f"tpB{j}")
                    for sc in range(GC):
                        nc.tensor.transpose(out=tp[:, sc * P:(sc + 1) * P],
                                            in_=msgT[j][:, sc * P:(sc + 1) * P],
                                            identity=ident[:])
                    if j == 0:
                        nc.vector.tensor_copy(
                            out=msg_gb[:, :, j * P:(j + 1) * P],
                            in_=tp[:].rearrange("p (c q) -> p c q", c=GC))
                    else:
                        nc.scalar.copy(
                            out=msg_gb[:, :, j * P:(j + 1) * P],
                            in_=tp[:].rearrange("p (c q) -> p c q", c=GC))
                for sc in range(GC):
                    ch = g * GC + sc
                    b = int(block_of_chunk[ch])
                    first = ch == int(base_b[b])
                    last = ch == int(base_b[b]) + int(K_b[b]) - 1
                    if first:
                        agg_ps[b] = ps_agg.tile([P, F], F32, tag="agg",
                                                name=f"aggps{b}")
                    ind = wk.tile([P, P], DT, tag="ind")
                    nc.vector.tensor_tensor(
                        out=ind[:], in0=iot[:],
                        in1=dl_sb[:, ch:ch + 1].to_broadcast([P, P]),
                        op=mybir.AluOpType.is_equal)
                    nc.tensor.matmul(out=agg_ps[b][:], lhsT=ind[:],
                                     rhs=msg_gb[:, sc], start=first, stop=last)
                    if last:
                        asb = aggpool.tile([P, F], F32, tag=f"aggsb{b}",
                                           name=f"aggsb{b}")
                        nc.vector.tensor_copy(out=asb[:], in_=agg_ps[b][:])
                        agg_sb[b] = asb
            ctxB2.__exit__(None, None, None)
            ctxB.__exit__(None, None, None)
            ctxE = tc.tile_pool(name="ps_end", bufs=1, space="PSUM")
            ps_end = ctxE.__enter__()

            # ---------------- BN2 stats
            pssum = [ps_end.tile([P, 1], F32, tag=f"es{c}", name=f"es{c}")
                     for c in range(FH)]
            pssq = [ps_end.tile([P, 1], F32, tag=f"eq{c}", name=f"eq{c}")
                    for c in range(FH)]
            for b in range(NB):
                sq = wk.tile([P, F], F32, tag="aggsq")
                nc.scalar.activation(sq[:], agg_sb[b][:],
                                     mybir.ActivationFunctionType.Square)
                for c in range(FH):
                    nc.tensor.matmul(out=pssum[c][:],
                                     lhsT=agg_sb[b][:, c * P:(c + 1) * P],
                                     rhs=ones_c[:], start=(b == 0),
                                     stop=(b == NB - 1))
                    nc.tensor.matmul(out=pssq[c][:],
                                     lhsT=sq[:, c * P:(c + 1) * P],
                                     rhs=ones_c[:], start=(b == 0),
                                     stop=(b == NB - 1))
            stg2 = wk.tile([P, 2 * FH], F32, tag="stg2")
            for c in range(FH):
                nc.vector.tensor_copy(out=stg2[:, c:c + 1], in_=pssum[c][:])
                nc.vector.tensor_copy(out=stg2[:, FH + c:FH + c + 1],
                                      in_=pssq[c][:])
            cc2_in = dram.tile([P, 2 * FH], F32, tag="cc2_in")
            cc2_out = dram.tile([P, 2 * FH], F32, tag="cc2_out")
            nc.gpsimd.dma_start(out=cc2_in[:], in_=stg2[:])
            if sim1:
                nc.gpsimd.dma_start(out=cc2_out[:], in_=cc2_in[:])
            else:
                nc.gpsimd.collective_compute(
                    "AllReduce", mybir.AluOpType.add,
                    replica_groups=[list(range(n_cores))],
                    ins=[cc2_in.opt()], outs=[cc2_out.opt()])
            st2_all = wk.tile([P, 2 * FH], F32, tag="st2_all")
            nc.sync.dma_start(out=st2_all[:], in_=cc2_out[:])

            al2, be2 = bn_finalize(st2_all, zero2, zero2,
                                   pr["gbn"], pr["bbn"], inv_N, "bn2")

            # broadcast alpha/beta rows to [P, 2F]
            ab = wk.tile([P, 2 * FH], F32, tag="ab")
            nc.vector.tensor_copy(out=ab[:, 0:FH], in_=al2[:])
            nc.vector.tensor_copy(out=ab[:, FH:2 * FH], in_=be2[:])
            abb_ps = ps_end.tile([P, 2 * F], F32, tag="abb")
            abT = []
            for mcol in range(2 * FH):
                abT_ps = ps_end.tile([1, P], F32, tag="abT", name=f"abT{mcol}")
                nc.tensor.transpose(out=abT_ps[:], in_=ab[:, mcol:mcol + 1],
                                    identity=ident32[:])
                abrow = wk.tile([1, P], F32, tag="abrow", name=f"abrow{mcol}")
                nc.vector.tensor_copy(out=abrow[:], in_=abT_ps[:])
                abT.append(abrow)
            for mcol in range(2 * FH):
                nc.tensor.matmul(out=abb_ps[:, mcol * P:(mcol + 1) * P],
                                 lhsT=ones_r[:], rhs=abT[mcol][:],
                                 start=True, stop=True)
            abb = wk.tile([P, 2 * F], F32, tag="abb_sb")
            nc.vector.tensor_copy(out=abb[:], in_=abb_ps[:])

            ctxE.__exit__(None, None, None)

            # ---------------- final output
            for b in range(NB):
                o = wk.tile([P, F], F32, tag="fin_o")
                nc.vector.tensor_tensor(out=o[:], in0=agg_sb[b][:],
                                        in1=abb[:, 0:F], op=mybir.AluOpType.mult)
                nc.vector.tensor_tensor(out=o[:], in0=o[:], in1=abb[:, F:2 * F],
                                        op=mybir.AluOpType.add)
                xr = wk.tile([P, F], F32, tag="fin_xr")
                nc.sync.dma_start(out=xr[:], in_=xres[b * P:(b + 1) * P, :])
                nc.vector.tensor_tensor(out=o[:], in0=o[:], in1=xr[:],
                                        op=mybir.AluOpType.add)
                o2 = wk.tile([P, F], F32, tag="fin_o2")
                nc.scalar.activation(o2[:], o[:],
                                     mybir.ActivationFunctionType.Relu)
                nc.sync.dma_start(out=out[b * P:(b + 1) * P, :], in_=o2[:])

    nc.compile()
    return nc


# ---------------------------------------------------------------- entry point
def kernel(x, edge_index, edge_attr, W1f, b1f, W2f, b2f,
           W1m, b1m, W2m, b2m, g_int, beta_int, g_bn, beta_bn,
           n_nodes=N_NODES, n_cores=N_CORES):
    x = np.asarray(x, np.float32)
    edge_attr = np.asarray(edge_attr, np.float32)
    # this build assumes zero MLP biases (true for this problem's inputs)
    for b_ in (b1f, b2f, b1m, b2m):
        assert not np.any(np.asarray(b_)), "nonzero MLP bias unsupported"
    per_core, x16, meta = _prep(x, edge_index, edge_attr, n_nodes, n_cores)

    key = (n_cores, n_nodes, meta["C"], tuple(int(k) for k in meta["K_b"]))
    if key not in _cache:
        _cache[key] = _build(meta, n_cores, n_nodes)
    nc = _cache[key]

    # pad-edge correction for BN1 stats (gp_pad == 0 with zero biases)
    n_pad = n_cores * meta["E_PAD"] - meta["E"]
    gp_pad = np.zeros(F, np.float64)
    corr1 = _col2(n_pad * gp_pad)
    corr2 = _col2(n_pad * gp_pad ** 2)

    iota_np = np.broadcast_to(np.arange(P, dtype=NPDT), (P, P)).copy()
    common = dict(
        xt=x16,
        w1f=np.asarray(W1f, np.float32).astype(NPDT),
        w2f=np.asarray(W2f, np.float32).astype(NPDT),
        w1m=np.asarray(W1m, np.float32).astype(NPDT),
        w2m=np.asarray(W2m, np.float32).astype(NPDT),
        gint=_col2(g_int), bint=_col2(beta_int),
        gbn=_col2(g_bn), bbn=_col2(beta_bn),
        corr1=corr1, corr2=corr2, iota=iota_np,
    )
    in_maps = []
    for c in range(n_cores):
        m = dict(common)
        m.update(per_core[c])
        in_maps.append(m)

    res = bass_utils.run_bass_kernel_spmd(nc, in_maps,
                                          core_ids=list(range(n_cores)))
    NPC = meta["NPC"]
    return np.concatenate([res.results[c]["out"][:NPC]
                           for c in range(n_cores)], axis=0)


# revision 16
# speedup vs baseline: 55.2155x; 55.2155x over previous
"""PotNet GNN message-passing kernel for 8 Trainium2 NeuronCores (Bass/Tile).

Strategy: edges sorted by dst and sharded so core c owns nodes
[c*2500, (c+1)*2500) and ALL edges targeting them (no cross-core aggregation
needed). Edge MLPs run locally in fp16 (weight-stationary matmuls on
transposed activations); the two BatchNorms need global batch stats, done
with tiny AllReduce collectives. Aggregation (segment_sum) is a one-hot
indicator matmul into per-node-block PSUM accumulators. The x[dst] gather is
avoided entirely: edges are grouped by 128-node blocks, so the dst
contribution of layer 1 is (x_blk @ W1a) expanded per-edge with the same
one-hot indicator (transposed) on the PE.
"""
import numpy as np

import concourse.bass as bass
import concourse.bacc as bacc
import concourse.mybir as mybir
import concourse.tile as tile
import concourse.bass_utils as bass_utils
from concourse.masks import make_identity


def _make_runner(nc, n_cores):
    """Persistent PJRT runner (jit once, reuse across calls)."""
    import jax
    from jax.sharding import Mesh, PartitionSpec, NamedSharding
    from jax.experimental.shard_map import shard_map
    from concourse.bass2jax import (_bass_exec_p, install_neuronx_cc_hook,
                                    partition_id_tensor)
    install_neuronx_cc_hook()
    pname = nc.partition_id_tensor.name if nc.partition_id_tensor else None
    in_names, out_names, out_avals, zero_outs = [], [], [], []
    for alloc in nc.m.functions[0].allocations:
        if not isinstance(alloc, mybir.MemoryLocationSet):
            continue
        name = alloc.memorylocations[0].name
        if alloc.kind == "ExternalInput":
            if name != pname:
                in_names.append(name)
        elif alloc.kind == "ExternalOutput":
            out_names.append(name)
            shape = tuple(alloc.tensor_shape)
            dtype = mybir.dt.np(alloc.dtype)
            out_avals.append(jax.core.ShapedArray(shape, dtype))
            zero_outs.append(np.zeros(shape, dtype))
    n_params = len(in_names)
    n_outs = len(out_avals)
    all_in = list(in_names) + list(out_names) + ([pname] if pname else [])

    def _body(*args):
        operands = list(args)
        if pname is not None:
            operands.append(partition_id_tensor())
        return tuple(_bass_exec_p.bind(
            *operands, out_avals=tuple(out_avals), in_names=tuple(all_in),
            out_names=tuple(out_names), lowering_input_output_aliases=(),
            sim_require_finite=True, sim_require_nnan=True, nc=nc))

    devices = jax.devices()[:n_cores]
    mesh = Mesh(np.asarray(devices), ("core",))
    in_specs = (PartitionSpec("core"),) * (n_params + n_outs)
    out_specs = (PartitionSpec("core"),) * n_outs
    donate = tuple(range(n_params, n_params + n_outs))
    sharded = jax.jit(
        shard_map(_body, mesh=mesh, in_specs=in_specs, out_specs=out_specs,
                  check_rep=False),
        donate_argnums=donate, keep_unused=True)
    sh = NamedSharding(mesh, PartitionSpec("core"))

    def run(in_maps):
        per_core = [[np.asarray(m[n]) for n in in_names] for m in in_maps]
        concat_in = [np.concatenate([per_core[c][i] for c in range(n_cores)], 0)
                     for i in range(n_params)]
        dev_in = [jax.device_put(a, sh) for a in concat_in]
        zeros = [jax.device_put(
            np.zeros((n_cores * z.shape[0], *z.shape[1:]), z.dtype), sh)
            for z in zero_outs]
        outs = sharded(*dev_in, *zeros)
        return [{name: np.asarray(outs[i]).reshape(n_cores, *out_avals[i].shape)[c]
                 for i, name in enumerate(out_names)}
                for c in range(n_cores)]

    run.device_run = sharded
    run.in_names = in_names
    run.zero_outs = zero_outs
    run.n_cores = n_cores
    run.sharding = sh
    return run

P = 128          # partitions / chunk size (edges)
F = 256          # feature dim
FH = 2           # feature halves (F // P)
GC = 4           # chunks per compute group
GROUP = P * GC   # 512 edges per group
EPS = 1e-5

N_NODES = 20000
N_CORES = 8

DT = mybir.dt.float16
F32 = mybir.dt.float32
NPDT = np.float16

_cache = {}


# ---------------------------------------------------------------- host prep
def _prep(x, edge_index, edge_attr, n_nodes, n_cores):
    E = edge_index.shape[1]
    NPC = n_nodes // n_cores          # nodes per core
    NB = -(-NPC // P)                 # node blocks per core
    src = np.asarray(edge_index[0], np.int64)
    dst = np.asarray(edge_index[1], np.int64)

    order = np.argsort(dst, kind="stable")
    dst_s = dst[order]
    src_s = src[order]

    core_of = dst_s // NPC
    nl = dst_s - core_of * NPC        # node-local id
    b_of = nl // P                    # block within core
    dloc = (nl - b_of * P)            # position within block

    gid = core_of * NB + b_of
    cnts = np.bincount(gid, minlength=n_cores * NB).reshape(n_cores, NB)
    K_b = np.maximum(1, -(-cnts.max(axis=0) // P)).astype(np.int64)  # chunks/block
    C0 = int(K_b.sum())
    K_b[NB - 1] += (-C0) % GC
    C = int(K_b.sum())
    G = C // GC
    base_b = np.zeros(NB, np.int64)
    base_b[1:] = np.cumsum(K_b)[:-1]
    E_PAD = C * P

    # rank of each edge within its (core, block) group
    group_starts = np.zeros(n_cores * NB + 1, np.int64)
    group_starts[1:] = np.cumsum(np.bincount(gid, minlength=n_cores * NB))
    rank = np.arange(E) - group_starts[gid]
    slot = (base_b[b_of] + rank // P) * P + rank % P   # slot within core's arrays

    Vx = n_nodes + 1
    x16 = np.vstack([np.asarray(x, np.float32).astype(NPDT),
                     np.zeros((1, F), NPDT)])          # row n_nodes = zeros
    ea16 = np.asarray(edge_attr, np.float32).astype(NPDT)[order]

    per_core = []
    for c in range(n_cores):
        m = core_of == c
        sl = slot[m]
        ea_sh = np.zeros((E_PAD, F), NPDT)
        ea_sh[sl] = ea16[m]
        row, col = sl % P, sl // P
        src_i = np.full((P, C), n_nodes, np.int32)
        dlc = np.full((P, C), -1.0, NPDT)
        src_i[row, col] = src_s[m]
        dlc[row, col] = dloc[m].astype(NPDT)
        xr = np.zeros((NB * P, F), np.float32)
        xr[:NPC] = np.asarray(x, np.float32)[c * NPC:(c + 1) * NPC]
        per_core.append(dict(ea=ea_sh, srci=src_i, dstloc=dlc,
                             xres=xr, xblk=xr.astype(NPDT)))
    meta = dict(E=E, NPC=NPC, NB=NB, C=C, G=G, Vx=Vx, E_PAD=E_PAD,
                K_b=K_b, base_b=base_b)
    return per_core, x16, meta


def _col2(v):
    """[F] vector -> [P, FH] f32, feature f = c*P + p at [p, c]."""
    return np.asarray(v, np.float32).reshape(FH, P).T.copy()


def _silu64(v):
    return v / (1.0 + np.exp(-v))


def _chunk_runs(chunks, block_of_chunk):
    """Split chunk ids into runs of equal block: [(b, i0, i1)]."""
    runs = []
    i0 = 0
    for i in range(1, len(chunks) + 1):
        if i == len(chunks) or \
                block_of_chunk[chunks[i]] != block_of_chunk[chunks[i0]]:
            runs.append((int(block_of_chunk[chunks[i0]]), i0, i))
            i0 = i
    return runs


# ---------------------------------------------------------------- device build
def _build(meta, n_cores, n_nodes, sim1=False, cfg=None):
    cfg = cfg or dict(mm=2, tp=4, wk=4, comb=4, agg=3)
    BL1 = cfg.get('l1', 2); BL2 = cfg.get('l2', 1); BTP = cfg.get('tp', 2)
    BMM = cfg.get('mm', 0)  # if set, share one psum pool/tag for l1+l2
    BWK = cfg.get('wk', 3); BCB = cfg.get('comb', 2)
    BAG = cfg.get('agg', 2)
    NB, C, G, Vx = meta["NB"], meta["C"], meta["G"], meta["Vx"]
    K_b, base_b = meta["K_b"], meta["base_b"]
    E_PAD, E = meta["E_PAD"], meta["E"]
    block_of_chunk = np.repeat(np.arange(NB), K_b)

    inv_E = 1.0 / float(E)
    inv_N = 1.0 / float(n_nodes)

    nc = bacc.Bacc("TRN2", target_bir_lowering=False, debug=False,
                   enable_asserts=True, num_devices=1 if sim1 else n_cores)

    ea = nc.dram_tensor("ea", [E_PAD, F], DT, kind="ExternalInput")
    srci = nc.dram_tensor("srci", [P, C], mybir.dt.int32, kind="ExternalInput")
    dstloc = nc.dram_tensor("dstloc", [P, C], DT, kind="ExternalInput")
    xt = nc.dram_tensor("xt", [Vx, F], DT, kind="ExternalInput")
    xres = nc.dram_tensor("xres", [NB * P, F], F32, kind="ExternalInput")
    xblk = nc.dram_tensor("xblk", [NB * P, F], DT, kind="ExternalInput")
    w1f = nc.dram_tensor("w1f", [3 * F, F], DT, kind="ExternalInput")
    w2f = nc.dram_tensor("w2f", [F, F], DT, kind="ExternalInput")
    w1m = nc.dram_tensor("w1m", [3 * F, F], DT, kind="ExternalInput")
    w2m = nc.dram_tensor("w2m", [F, F], DT, kind="ExternalInput")
    pnames = ["gint", "bint", "gbn", "bbn", "corr1", "corr2"]
    pdram = {n: nc.dram_tensor(n, [P, FH], F32, kind="ExternalInput")
             for n in pnames}
    iota = nc.dram_tensor("iota", [P, P], DT, kind="ExternalInput")
    out = nc.dram_tensor("out", [NB * P, F], F32, kind="ExternalOutput")

    SCR = 4 * GROUP   # combined scratch row: [gp0|gp1|mp0|mp1]

    with tile.TileContext(nc) as tc:
        with tc.tile_pool(name="const", bufs=1) as cst, \
             tc.tile_pool(name="work", bufs=BWK) as wk, \
             tc.tile_pool(name="aggsb", bufs=1) as aggpool, \
             tc.tile_pool(name="dram", bufs=1, space="DRAM") as dram:

            # ---------------- resident constants
            w1t = {}
            w2t = {}
            for nm, drt in (("f", w1f), ("m", w1m)):
                w1t[nm] = []
                for k in range(6):
                    t = cst.tile([P, F], DT, tag=f"w1{nm}{k}", name=f"w1{nm}{k}")
                    nc.sync.dma_start(out=t[:], in_=drt[k * P:(k + 1) * P, :])
                    w1t[nm].append(t)
            for nm, drt in (("f", w2f), ("m", w2m)):
                w2t[nm] = []
                for k in range(FH):
                    t = cst.tile([P, F], DT, tag=f"w2{nm}{k}", name=f"w2{nm}{k}")
                    nc.sync.dma_start(out=t[:], in_=drt[k * P:(k + 1) * P, :])
                    w2t[nm].append(t)
            pr = {}
            for n in pnames:
                t = cst.tile([P, FH], F32, tag=f"p_{n}", name=f"p_{n}")
                nc.sync.dma_start(out=t[:], in_=pdram[n][:])
                pr[n] = t
            iot = cst.tile([P, P], DT, tag="iota")
            nc.sync.dma_start(out=iot[:], in_=iota[:])
            ident = cst.tile([P, P], DT, tag="ident")
            make_identity(nc, ident[:])
            ident32 = cst.tile([P, P], F32, tag="ident32")
            make_identity(nc, ident32[:])
            dl_sb = cst.tile([P, C], DT, tag="dstloc")
            nc.sync.dma_start(out=dl_sb[:], in_=dstloc[:])
            si_sb = cst.tile([P, C], mybir.dt.int32, tag="srci")
            nc.sync.dma_start(out=si_sb[:], in_=srci[:])
            ones_c = cst.tile([P, 1], F32, tag="ones_c")
            nc.gpsimd.memset(ones_c[:], 1.0)
            ones_r = cst.tile([1, P], F32, tag="ones_r")
            nc.gpsimd.memset(ones_r[:], 1.0)
            s1 = cst.tile([P, FH], F32, tag="s1")
            nc.gpsimd.memset(s1[:], 0.0)
            s2 = cst.tile([P, FH], F32, tag="s2")
            nc.gpsimd.memset(s2[:], 0.0)

            scr = dram.tile([G, P, SCR], DT, tag="scr")

            # ---------------- PASS A
            if BMM:
                ctxA = tc.tile_pool(name="ps_mm", bufs=BMM, space="PSUM")
                ps_l1 = ctxA.__enter__()
                ctxA2 = None
                ps_l2 = ps_l1
            else:
                ctxA = tc.tile_pool(name="ps_l1", bufs=BL1, space="PSUM")
                ps_l1 = ctxA.__enter__()
                ctxA2 = tc.tile_pool(name="ps_l2", bufs=BL2, space="PSUM")
                ps_l2 = ctxA2.__enter__()
            ctxA3 = tc.tile_pool(name="ps_tp", bufs=BTP, space="PSUM")
            ps_tp = ctxA3.__enter__()

            # lazily-computed per-block xW1a tables (dst part of layer 1)
            xw1a = {"f": [None] * NB, "m": [None] * NB}

            def ensure_xw1a(b):
                if xw1a["f"][b] is not None:
                    return
                xb = wk.tile([P, F], DT, tag="xb", name=f"xb{b}")
                nc.sync.dma_start(out=xb[:], in_=xblk[b * P:(b + 1) * P, :])
                tpx = ps_tp.tile([P, FH * P], DT, tag="tp", name=f"tpx{b}")
                for kc in range(FH):
                    nc.tensor.transpose(out=tpx[:, kc * P:(kc + 1) * P],
                                        in_=xb[:, kc * P:(kc + 1) * P],
                                        identity=ident[:])
                xbT = wk.tile([P, FH, P], DT, tag="xbT", name=f"xbT{b}")
                nc.vector.tensor_copy(
                    out=xbT[:], in_=tpx[:].rearrange("p (c q) -> p c q", c=FH))
                for br in ("f", "m"):
                    psx = ps_l1.tile([P, F], F32, tag="mm" if BMM else "l1",
                                     name=f"psx{b}{br}")
                    for kc in range(FH):
                        nc.tensor.matmul(out=psx[:], lhsT=xbT[:, kc],
                                         rhs=w1t[br][kc][:],
                                         start=(kc == 0), stop=(kc == FH - 1))
                    xw = wk.tile([P, F], DT, tag=f"xw1a{br}", name=f"xw{b}{br}")
                    nc.vector.tensor_copy(out=xw[:], in_=psx[:])
                    xw1a[br][b] = xw

            for g in range(G):
                e0 = g * GROUP
                chunks = list(range(g * GC, (g + 1) * GC))
                runs = _chunk_runs(chunks, block_of_chunk)
                for b, _, _ in runs:
                    ensure_xw1a(b)

                # x[src] gather + transpose
                gxs = wk.tile([P, GC, F], DT, tag="g_xs")
                for sc in range(GC):
                    nc.gpsimd.indirect_dma_start(
                        out=gxs[:, sc], out_offset=None, in_=xt[:],
                        in_offset=bass.IndirectOffsetOnAxis(
                            ap=si_sb[:, g * GC + sc:g * GC + sc + 1], axis=0))
                hT = []
                for h in range(FH):
                    tp = ps_tp.tile([P, GROUP], DT, tag="tp", name=f"tpxs{h}")
                    for sc in range(GC):
                        nc.tensor.transpose(
                            out=tp[:, sc * P:(sc + 1) * P],
                            in_=gxs[:, sc, h * P:(h + 1) * P],
                            identity=ident[:])
                    tT = wk.tile([P, GROUP], DT, tag=f"T_xs{h}", name=f"T_xs{h}")
                    nc.vector.tensor_copy(out=tT[:], in_=tp[:])
                    hT.append(tT)
                for h in range(FH):
                    tT = wk.tile([P, GROUP], DT, tag=f"T_ea{h}", name=f"T_ea{h}")
                    nc.sync.dma_start(out=tT[:],
                                      in_=ea[e0:e0 + GROUP, h * P:(h + 1) * P],
                                      transpose=True)
                    hT.append(tT)

                # indicator (edge-part) + transposed version for dst expansion
                ind_g = wk.tile([P, GC, P], DT, tag="ind_g")
                for sc in range(GC):
                    nc.vector.tensor_tensor(
                        out=ind_g[:, sc], in0=iot[:],
                        in1=dl_sb[:, g * GC + sc:g * GC + sc + 1].to_broadcast([P, P]),
                        op=mybir.AluOpType.is_equal)
                tpi = ps_tp.tile([P, GROUP], DT, tag="tp", name="tpi")
                for sc in range(GC):
                    nc.tensor.transpose(out=tpi[:, sc * P:(sc + 1) * P],
                                        in_=ind_g[:, sc], identity=ident[:])
                indT = wk.tile([P, GROUP], DT, tag="indT")
                nc.vector.tensor_copy(out=indT[:], in_=tpi[:])

                comb = wk.tile([P, SCR], DT, tag="comb", bufs=BCB)
                for bi_, br in enumerate(("f", "m")):
                    ps = ps_l1.tile([P, 2 * GROUP], F32, tag="mm" if BMM else "l1",
                                    name=f"l1{br}")
                    # src + ea contributions first (no indT dependency)
                    for k in range(4):
                        for j in range(FH):
                            nc.tensor.matmul(
                                out=ps[:, j * GROUP:(j + 1) * GROUP],
                                lhsT=w1t[br][2 + k][:, j * P:(j + 1) * P],
                                rhs=hT[k][:], start=(k == 0), stop=False,
                                skip_group_check=True)
                    # dst contribution via indicator expansion (last)
                    for b, i0, i1 in runs:
                        for j in range(FH):
                            nc.tensor.matmul(
                                out=ps[:, j * GROUP + i0 * P:j * GROUP + i1 * P],
                                lhsT=xw1a[br][b][:, j * P:(j + 1) * P],
                                rhs=indT[:, i0 * P:i1 * P],
                                start=False, stop=True, skip_group_check=True)
                    t1 = wk.tile([P, 2 * GROUP], DT, tag=f"t1{br}",
                                 name=f"t1{br}")
                    for k in range(FH):
                        nc.scalar.activation(
                            t1[:, k * GROUP:(k + 1) * GROUP],
                            ps[:, k * GROUP:(k + 1) * GROUP],
                            mybir.ActivationFunctionType.Silu)
                    ps2 = ps_l2.tile([P, 2 * GROUP], F32, tag="mm" if BMM else "l2",
                                     name=f"l2{br}")
                    for k in range(FH):
                        for j in range(FH):
                            nc.tensor.matmul(
                                out=ps2[:, j * GROUP:(j + 1) * GROUP],
                                lhsT=w2t[br][k][:, j * P:(j + 1) * P],
                                rhs=t1[:, k * GROUP:(k + 1) * GROUP],
                                start=(k == 0), stop=(k == FH - 1))
                    if br == "f":
                        # evict per half with running-sum accumulation (ACT);
                        # sumsq on DVE from the fp16 gp values
                        for j in range(FH):
                            r1 = wk.tile([P, 1], F32, tag=f"r1_{j}",
                                         name=f"r1_{j}")
                            nc.scalar.activation(
                                comb[:, j * GROUP:(j + 1) * GROUP],
                                ps2[:, j * GROUP:(j + 1) * GROUP],
                                mybir.ActivationFunctionType.Identity,
                                accum_out=r1[:])
                            nc.vector.tensor_tensor(
                                out=s1[:, j:j + 1], in0=s1[:, j:j + 1],
                                in1=r1[:], op=mybir.AluOpType.add)
                        sqt = wk.tile([P, 2 * GROUP], DT, tag="sqt", bufs=2)
                        nc.vector.tensor_tensor(
                            out=sqt[:], in0=comb[:, 0:2 * GROUP],
                            in1=comb[:, 0:2 * GROUP], op=mybir.AluOpType.mult)
                        r2 = wk.tile([P, FH], F32, tag="r2")
                        nc.vector.tensor_reduce(
                            out=r2[:],
                            in_=sqt[:].rearrange("p (c q) -> p c q", c=FH),
                            op=mybir.AluOpType.add, axis=mybir.AxisListType.X)
                        nc.vector.tensor_tensor(out=s2[:], in0=s2[:],
                                                in1=r2[:],
                                                op=mybir.AluOpType.add)
                    else:
                        nc.scalar.activation(
                            comb[:, bi_ * 2 * GROUP:(bi_ + 1) * 2 * GROUP],
                            ps2[:],
                            mybir.ActivationFunctionType.Identity)
                nc.scalar.dma_start(out=scr[g], in_=comb[:])

            ctxA3.__exit__(None, None, None)
            if ctxA2 is not None:
                ctxA2.__exit__(None, None, None)
            ctxA.__exit__(None, None, None)

            # ---------------- BN1 stats all-reduce + finalize
            stg = wk.tile([P, 2 * FH], F32, tag="stg")
            nc.vector.tensor_copy(out=stg[:, 0:FH], in_=s1[:])
            nc.vector.tensor_copy(out=stg[:, FH:2 * FH], in_=s2[:])
            cc1_in = dram.tile([P, 2 * FH], F32, tag="cc1_in")
            cc1_out = dram.tile([P, 2 * FH], F32, tag="cc1_out")
            nc.gpsimd.dma_start(out=cc1_in[:], in_=stg[:])
            if sim1:
                nc.gpsimd.dma_start(out=cc1_out[:], in_=cc1_in[:])
            else:
                nc.gpsimd.collective_compute(
                    "AllReduce", mybir.AluOpType.add,
                    replica_groups=[list(range(n_cores))],
                    ins=[cc1_in.opt()], outs=[cc1_out.opt()])
            st_all = wk.tile([P, 2 * FH], F32, tag="st_all")
            nc.sync.dma_start(out=st_all[:], in_=cc1_out[:])

            def bn_finalize(st_ap, corr1, corr2, gam, bet, inv_n, tag):
                """-> (alpha [P,FH], beta [P,FH]) f32 tiles."""
                mean = wk.tile([P, FH], F32, tag=f"{tag}mean", name=f"{tag}mean")
                nc.vector.tensor_tensor(out=mean[:], in0=st_ap[:, 0:FH],
                                        in1=corr1[:], op=mybir.AluOpType.subtract)
                nc.vector.tensor_scalar_mul(mean[:], mean[:], inv_n)
                ex2 = wk.tile([P, FH], F32, tag=f"{tag}ex2", name=f"{tag}ex2")
                nc.vector.tensor_tensor(out=ex2[:], in0=st_ap[:, FH:2 * FH],
                                        in1=corr2[:], op=mybir.AluOpType.subtract)
                nc.vector.tensor_scalar_mul(ex2[:], ex2[:], inv_n)
                var = wk.tile([P, FH], F32, tag=f"{tag}var", name=f"{tag}var")
                nc.vector.tensor_tensor(out=var[:], in0=mean[:], in1=mean[:],
                                        op=mybir.AluOpType.mult)
                nc.vector.tensor_tensor(out=var[:], in0=ex2[:], in1=var[:],
                                        op=mybir.AluOpType.subtract)
                nc.vector.tensor_scalar_add(var[:], var[:], float(EPS))
                sd = wk.tile([P, FH], F32, tag=f"{tag}sd", name=f"{tag}sd")
                nc.scalar.activation(sd[:], var[:],
                                     mybir.ActivationFunctionType.Sqrt)
                rs = wk.tile([P, FH], F32, tag=f"{tag}rs", name=f"{tag}rs")
                nc.vector.reciprocal(rs[:], sd[:])
                al = wk.tile([P, FH], F32, tag=f"{tag}al", name=f"{tag}al")
                nc.vector.tensor_tensor(out=al[:], in0=gam[:], in1=rs[:],
                                        op=mybir.AluOpType.mult)
                be = wk.tile([P, FH], F32, tag=f"{tag}be", name=f"{tag}be")
                nc.vector.tensor_tensor(out=be[:], in0=al[:], in1=mean[:],
                                        op=mybir.AluOpType.mult)
                nc.vector.tensor_tensor(out=be[:], in0=bet[:], in1=be[:],
                                        op=mybir.AluOpType.subtract)
                return al, be

            al1, be1 = bn_finalize(st_all, pr["corr1"], pr["corr2"],
                                   pr["gint"], pr["bint"], inv_E, "bn1")

            # ---------------- PASS B
            zero2 = cst.tile([P, FH], F32, tag="zero2")
            nc.gpsimd.memset(zero2[:], 0.0)
            agg_sb = [None] * NB
            agg_ps = {}
            ctxB = tc.tile_pool(name="ps_tpB", bufs=2, space="PSUM")
            ps_tpB = ctxB.__enter__()
            ctxB2 = tc.tile_pool(name="ps_agg", bufs=BAG, space="PSUM")
            ps_agg = ctxB2.__enter__()
            for g in range(G):
                comb_in = wk.tile([P, SCR], DT, tag="combB", bufs=BCB)
                nc.sync.dma_start(out=comb_in[:], in_=scr[g])
                msgT = []
                for j in range(FH):
                    gt = wk.tile([P, GROUP], DT, tag=f"gate_{j}",
                                 name=f"gate{j}")
                    nc.scalar.activation(
                        gt[:], comb_in[:, j * GROUP:(j + 1) * GROUP],
                        mybir.ActivationFunctionType.Sigmoid,
                        bias=be1[:, j:j + 1], scale=al1[:, j:j + 1])
                    mt = wk.tile([P, GROUP], DT, tag=f"msgT_{j}",
                                 name=f"msgT{j}")
                    nc.vector.tensor_tensor(
                        out=mt[:], in0=gt[:],
                        in1=comb_in[:, (2 + j) * GROUP:(3 + j) * GROUP],
                        op=mybir.AluOpType.mult)
                    msgT.append(mt)
                msg_gb = wk.tile([P, GC, F], DT, tag="msg_gb")
                for j in range(FH):
                    tp = ps_tpB.tile([P, GROUP], DT, tag="tpB", name=f"tpB{j}")
                    for sc in range(GC):
                        nc.tensor.transpose(out=tp[:, sc * P:(sc + 1) * P],
                                            in_=msgT[j][:, sc * P:(sc + 1) * P],
                                            identity=ident[:])
                    if j == 0:
                        nc.vector.tensor_copy(
                            out=msg_gb[:, :, j * P:(j + 1) * P],
                            in_=tp[:].rearrange("p (c q) -> p c q", c=GC))
                    else:
                        nc.scalar.copy(
                            out=msg_gb[:, :, j * P:(j + 1) * P],
                            in_=tp[:].rearrange("p (c q) -> p c q", c=GC))
                for sc in range(GC):
                    ch = g * GC + sc
                    b = int(block_of_chunk[ch])
                    first = ch == int(base_b[b])
                    last = ch == int(base_b[b]) + int(K_b[b]) - 1
                    if first:
                        agg_ps[b] = ps_agg.tile([P, F], F32, tag="agg",
                                                name=f"aggps{b}")
                    ind = wk.tile([P, P], DT, tag="ind")
                    nc.vector.tensor_tensor(
                        out=ind[:], in0=iot[:],
                        in1=dl_sb[:, ch:ch + 1].to_broadcast([P, P]),
                        op=mybir.AluOpType.is_equal)
                    nc.tensor.matmul(out=agg_ps[b][:], lhsT=ind[:],
                                     rhs=msg_gb[:, sc], start=first, stop=last)
                    if last:
                        asb = aggpool.tile([P, F], F32, tag=f"aggsb{b}",
                                           name=f"aggsb{b}")
                        nc.vector.tensor_copy(out=asb[:], in_=agg_ps[b][:])
                        agg_sb[b] = asb
            ctxB2.__exit__(None, None, None)
            ctxB.__exit__(None, None, None)
            ctxE = tc.tile_pool(name="ps_end", bufs=1, space="PSUM")
            ps_end = ctxE.__enter__()

            # ---------------- BN2 stats
            pssum = [ps_end.tile([P, 1], F32, tag=f"es{c}", name=f"es{c}")
                     for c in range(FH)]
            pssq = [ps_end.tile([P, 1], F32, tag=f"eq{c}", name=f"eq{c}")
                    for c in range(FH)]
            for b in range(NB):
                sq = wk.tile([P, F], F32, tag="aggsq")
                nc.scalar.activation(sq[:], agg_sb[b][:],
                                     mybir.ActivationFunctionType.Square)
                for c in range(FH):
                    nc.tensor.matmul(out=pssum[c][:],
                                     lhsT=agg_sb[b][:, c * P:(c + 1) * P],
                                     rhs=ones_c[:], start=(b == 0),
                                     stop=(b == NB - 1))
                    nc.tensor.matmul(out=pssq[c][:],
                                     lhsT=sq[:, c * P:(c + 1) * P],
                                     rhs=ones_c[:], start=(b == 0),
                                     stop=(b == NB - 1))
            stg2 = wk.tile([P, 2 * FH], F32, tag="stg2")
            for c in range(FH):
                nc.vector.tensor_copy(out=stg2[:, c:c + 1], in_=pssum[c][:])
                nc.vector.tensor_copy(out=stg2[:, FH + c:FH + c + 1],
                                      in_=pssq[c][:])
            cc2_in = dram.tile([P, 2 * FH], F32, tag="cc2_in")
            cc2_out = dram.tile([P, 2 * FH], F32, tag="cc2_out")
            nc.gpsimd.dma_start(out=cc2_in[:], in_=stg2[:])
            if sim1:
                nc.gpsimd.dma_start(out=cc2_out[:], in_=cc2_in[:])
            else:
                nc.gpsimd.collective_compute(
                    "AllReduce", mybir.AluOpType.add,
                    replica_groups=[list(range(n_cores))],
                    ins=[cc2_in.opt()], outs=[cc2_out.opt()])
            st2_all = wk.tile([P, 2 * FH], F32, tag="st2_all")
            nc.sync.dma_start(out=st2_all[:], in_=cc2_out[:])

            al2, be2 = bn_finalize(st2_all, zero2, zero2,
                                   pr["gbn"], pr["bbn"], inv_N, "bn2")

            # broadcast alpha/beta rows to [P, 2F]
            ab = wk.tile([P, 2 * FH], F32, tag="ab")
            nc.vector.tensor_copy(out=ab[:, 0:FH], in_=al2[:])
            nc.vector.tensor_copy(out=ab[:, FH:2 * FH], in_=be2[:])
            abb_ps = ps_end.tile([P, 2 * F], F32, tag="abb")
            abT = []
            for mcol in range(2 * FH):
                abT_ps = ps_end.tile([1, P], F32, tag="abT", name=f"abT{mcol}")
                nc.tensor.transpose(out=abT_ps[:], in_=ab[:, mcol:mcol + 1],
                                    identity=ident32[:])
                abrow = wk.tile([1, P], F32, tag="abrow", name=f"abrow{mcol}")
                nc.vector.tensor_copy(out=abrow[:], in_=abT_ps[:])
                abT.append(abrow)
            for mcol in range(2 * FH):
                nc.tensor.matmul(out=abb_ps[:, mcol * P:(mcol + 1) * P],
                                 lhsT=ones_r[:], rhs=abT[mcol][:],
                                 start=True, stop=True)
            abb = wk.tile([P, 2 * F], F32, tag="abb_sb")
            nc.vector.tensor_copy(out=abb[:], in_=abb_ps[:])

            ctxE.__exit__(None, None, None)

            # ---------------- final output
            for b in range(NB):
                o = wk.tile([P, F], F32, tag="fin_o")
                nc.vector.tensor_tensor(out=o[:], in0=agg_sb[b][:],
                                        in1=abb[:, 0:F], op=mybir.AluOpType.mult)
                nc.vector.tensor_tensor(out=o[:], in0=o[:], in1=abb[:, F:2 * F],
                                        op=mybir.AluOpType.add)
                xr = wk.tile([P, F], F32, tag="fin_xr")
                nc.sync.dma_start(out=xr[:], in_=xres[b * P:(b + 1) * P, :])
                nc.vector.tensor_tensor(out=o[:], in0=o[:], in1=xr[:],
                                        op=mybir.AluOpType.add)
                o2 = wk.tile([P, F], F32, tag="fin_o2")
                nc.scalar.activation(o2[:], o[:],
                                     mybir.ActivationFunctionType.Relu)
                nc.sync.dma_start(out=out[b * P:(b + 1) * P, :], in_=o2[:])

    nc.compile()
    return nc


# ---------------------------------------------------------------- entry point
def kernel(x, edge_index, edge_attr, W1f, b1f, W2f, b2f,
           W1m, b1m, W2m, b2m, g_int, beta_int, g_bn, beta_bn,
           n_nodes=N_NODES, n_cores=N_CORES):
    x = np.asarray(x, np.float32)
    edge_attr = np.asarray(edge_attr, np.float32)
    # this build assumes zero MLP biases (true for this problem's inputs)
    for b_ in (b1f, b2f, b1m, b2m):
        assert not np.any(np.asarray(b_)), "nonzero MLP bias unsupported"
    per_core, x16, meta = _prep(x, edge_index, edge_attr, n_nodes, n_cores)

    key = (n_cores, n_nodes, meta["C"], tuple(int(k) for k in meta["K_b"]))
    if key not in _cache:
        nc = _build(meta, n_cores, n_nodes)
        _cache[key] = (nc, _make_runner(nc, n_cores))
    nc, runner = _cache[key]

    # pad-edge correction for BN1 stats (gp_pad == 0 with zero biases)
    n_pad = n_cores * meta["E_PAD"] - meta["E"]
    gp_pad = np.zeros(F, np.float64)
    corr1 = _col2(n_pad * gp_pad)
    corr2 = _col2(n_pad * gp_pad ** 2)

    iota_np = np.broadcast_to(np.arange(P, dtype=NPDT), (P, P)).copy()
    common = dict(
        xt=x16,
        w1f=np.asarray(W1f, np.float32).astype(NPDT),
        w2f=np.asarray(W2f, np.float32).astype(NPDT),
        w1m=np.asarray(W1m, np.float32).astype(NPDT),
        w2m=np.asarray(W2m, np.float32).astype(NPDT),
        gint=_col2(g_int), bint=_col2(beta_int),
        gbn=_col2(g_bn), bbn=_col2(beta_bn),
        corr1=corr1, corr2=corr2, iota=iota_np,
    )
    in_maps = []
    for c in range(n_cores):
        m = dict(common)
        m.update(per_core[c])
        in_maps.append(m)

    results = runner(in_maps)
    NPC = meta["NPC"]
    return np.concatenate([results[c]["out"][:NPC]
                           for c in range(n_cores)], axis=0)
